# revision 5
# baseline (speedup 1.0000x reference)
"""Trainium2 Bass kernel for ClassicalSelfAttention.

  out = softmax((x @ Wq) @ (x @ Wk)^T / sqrt(D)) @ x      x: [8192, 1024] f32

Sharding (8 NeuronCores): rows of x are sharded across cores; each core
projects its own row-shard to Q^T and K^T, the K^T shards are AllGathered
across cores in two halves along the key dim (each fired as soon as its
half of the K projection finishes, so the collective overlaps the rest of
the projection and the own-block compute), and each core runs a streaming
attention loop over 16 key-blocks of 512 keys: scores matmul -> exp on
ScalarE -> PV matmul accumulated in SBUF. The softmax division is folded
into the final output scale. 1/sqrt(1024) = 2^-5 is folded into Wq on the
host (exact). All matmuls run in bf16 with fp32 PSUM accumulation
(measured end-to-end rel err ~4e-3, well within tolerance); bf16 halves
the input DMA so the PE starts sooner.

The scores matmul keeps K^T stationary and Q^T moving, so PSUM holds
scores TRANSPOSED ([key, query]); exp of that is P^T directly -- exactly
the layout the PV matmul needs as its stationary operand -- so no PE
transposes are needed. The softmax row-sums (a partition-dim reduction in
this layout) are computed by first summing the four 128-key chunks on the
Vector engine and then a single ones-vector matmul per query half (2
matmuls per block instead of 8, saving ~25us of PE time); the per-block
emission order (scores, sums_h0, PV mb0, PV mb1, sums_h1, PV mb2..7)
keeps the PE from ever waiting on the Vector-engine chunk sums.

Block processing order: each core processes its OWN two key blocks first
straight out of SBUF (plus its own V rows from a per-core x_shard input),
then the half-0 key blocks of peers (rank+1..rank+7, via dynamic DMA
offsets), then the half-1 blocks; softmax over key blocks is
order-invariant so any order works. This gives the half-0 AllGather
~150us of compute cover and the half-1 AllGather ~380us.

The final division by the softmax sums is fused into the last key block:
the sums transposes/reciprocal run on PE/Vector between that block's PV
matmuls and each query-block's scaled output (bf16) DMAs out while the
PE is still working on later query blocks, removing the serial tail.
"""

import sys

import numpy as np

try:
    import concourse.bass as bass  # noqa: F401
except ImportError:  # pragma: no cover
    sys.path.insert(0, "/opt/trn_rl_repo")

import concourse.bacc as bacc
import concourse.mybir as mybir
import concourse.tile as tile
from concourse.masks import make_identity
from concourse import bass_utils
from concourse.bass import ds

N_TOKENS = 8192
EMBED = 1024
NCORES = 8
M = N_TOKENS // NCORES  # rows per core (1024)
P = 128  # partitions
DC = EMBED // P  # contraction chunks (8)
NB = 512  # key-block width
NNB = N_TOKENS // NB  # key blocks (16)
MB = M // P  # query row-blocks per core (8)
VC = NB // P  # value chunks per key block (4)
HPR = M // NB  # key-block halves per rank (2)
FP32 = mybir.dt.float32
BF16 = mybir.dt.bfloat16
EXP = mybir.ActivationFunctionType.Exp
ADD = mybir.AluOpType.add
AXX = mybir.AxisListType.X


def _build():
    nc = bacc.Bacc(
        "TRN2", target_bir_lowering=False, debug=False, num_devices=NCORES
    )
    xt_shard = nc.dram_tensor("xt_shard", [EMBED, M], BF16, kind="ExternalInput").ap()
    x_shard = nc.dram_tensor("x_shard", [M, EMBED], BF16, kind="ExternalInput").ap()
    x_full = nc.dram_tensor(
        "x_full", [N_TOKENS, EMBED], BF16, kind="ExternalInput"
    ).ap()
    wq_d = nc.dram_tensor("wq", [EMBED, EMBED], BF16, kind="ExternalInput").ap()
    wk_d = nc.dram_tensor("wk", [EMBED, EMBED], BF16, kind="ExternalInput").ap()
    out_d = nc.dram_tensor("out", [M, EMBED], BF16, kind="ExternalOutput").ap()

    wq_r = wq_d.rearrange("(a p) d -> a p d", p=P)  # [DC, P, EMBED]
    wk_r = wk_d.rearrange("(a p) d -> a p d", p=P)
    xt_r = xt_shard.rearrange("(a p) m -> a p m", p=P)  # [DC, P, M]
    xs_r = x_shard.rearrange("(t p) d -> t p d", p=P)  # [M//P, P, EMBED]
    xv_r = x_full.rearrange("(t p) d -> t p d", p=P)  # [64, P, EMBED]
    out_r = out_d.rearrange("(t p) d -> t p d", p=P)  # [MB, P, EMBED]

    with tile.TileContext(nc) as tc:
        with (
            tc.tile_pool(name="persist", bufs=1) as pers,
            tc.tile_pool(name="persist_dram", bufs=1, space="DRAM") as pdram,
        ):
            ones_sb = pers.tile([P, P], BF16)
            nc.vector.memset(ones_sb[:], 1.0)
            ident = pers.tile([P, P], FP32)
            make_identity(nc, ident[:])
            # Q^T resident for the whole kernel: qt[p, b*M + m] = Qt[b*128+p, m]
            qt = pers.tile([P, DC * M], BF16)
            # own K^T shard, kept resident: ktsb[p, b*M + n] = Kt_own[b*128+p, n]
            ktsb = pers.tile([P, DC * M], BF16)
            # fp32 PV accumulator per query block: [p, mb*EMBED + dv]
            out_acc = pers.tile([P, MB * EMBED], FP32)
            # softmax denominators, replicated across partitions: [p, m]
            sums_acc = pers.tile([P, M], FP32)
            # per-query reciprocal denominators: [p, mb] for query mb*128+p
            scol = pers.tile([P, MB], FP32)
            rtot = pers.tile([P, MB], FP32)
            # K^T shard halves (AllGather inputs) and gathered halves.
            # ktd_h[j][b] = own K^T rows b*128..(b+1)*128-1, keys half j.
            # gkt_h[j][r*DC + b] = same for rank r.
            ktd_h = [
                pdram.tile([DC, P, NB], BF16, name=f"ktd{j}", tag=f"ktd{j}")
                for j in range(HPR)
            ]
            gkt_h = [
                pdram.tile(
                    [NCORES * DC, P, NB],
                    BF16,
                    addr_space="Shared",
                    name=f"gkt{j}",
                    tag=f"gkt{j}",
                )
                for j in range(HPR)
            ]

            rank = nc.gpsimd.partition_id()

            # ---- Phase A: project K^T shard (AllGather each key half as it
            # completes), then Q^T (own rows).
            with (
                tc.tile_pool(name="proj", bufs=1) as proj,
                tc.tile_pool(name="proj_ps", bufs=4, space="PSUM") as proj_ps,
            ):
                wq_sb = proj.tile([P, DC * EMBED], BF16)
                wk_sb = proj.tile([P, DC * EMBED], BF16)
                xt_sb = proj.tile([P, DC * M], BF16)
                # DMA priority: wk+xt (needed first), then wq.
                for a in range(DC):
                    nc.sync.dma_start(
                        out=wk_sb[:, a * EMBED : (a + 1) * EMBED], in_=wk_r[a]
                    )
                    nc.sync.dma_start(
                        out=xt_sb[:, a * M : (a + 1) * M], in_=xt_r[a]
                    )
                for a in range(DC):
                    nc.sync.dma_start(
                        out=wq_sb[:, a * EMBED : (a + 1) * EMBED], in_=wq_r[a]
                    )
                # K^T first so its AllGathers overlap the Q^T projection;
                # key-half j outer so each half's gather fires at the
                # halfway point of the K projection.
                for j in range(HPR):
                    for b in range(DC):  # output dim chunk
                        ps = proj_ps.tile([P, NB], FP32, tag="proj_ps")
                        for a in range(DC):  # contraction chunk
                            nc.tensor.matmul(
                                ps[:],
                                lhsT=wk_sb[:, a * EMBED + b * P : a * EMBED + (b + 1) * P],
                                rhs=xt_sb[:, a * M + j * NB : a * M + (j + 1) * NB],
                                start=(a == 0),
                                stop=(a == DC - 1),
                            )
                        nc.vector.tensor_copy(
                            out=ktsb[:, b * M + j * NB : b * M + (j + 1) * NB],
                            in_=ps[:],
                        )
                        nc.sync.dma_start(
                            out=ktd_h[j][b],
                            in_=ktsb[:, b * M + j * NB : b * M + (j + 1) * NB],
                        )
                    nc.gpsimd.collective_compute(
                        "AllGather",
                        mybir.AluOpType.bypass,
                        replica_groups=[list(range(NCORES))],
                        ins=[ktd_h[j].opt()],
                        outs=[gkt_h[j].opt()],
                    )
                for b in range(DC):
                    for j in range(HPR):
                        ps = proj_ps.tile([P, NB], FP32, tag="proj_ps")
                        for a in range(DC):
                            nc.tensor.matmul(
                                ps[:],
                                lhsT=wq_sb[:, a * EMBED + b * P : a * EMBED + (b + 1) * P],
                                rhs=xt_sb[:, a * M + j * NB : a * M + (j + 1) * NB],
                                start=(a == 0),
                                stop=(a == DC - 1),
                            )
                        nc.vector.tensor_copy(
                            out=qt[:, b * M + j * NB : b * M + (j + 1) * NB],
                            in_=ps[:],
                        )

            # ---- Phase B: streaming attention over key blocks.
            # Own two blocks first (K^T already in SBUF), then peers'
            # half-0 blocks, then peers' half-1 blocks.
            order = [(0, h) for h in range(HPR)] + [
                (j, h) for h in range(HPR) for j in range(1, NCORES)
            ]
            with (
                tc.tile_pool(name="kv", bufs=2) as kvp,
                tc.tile_pool(name="pb", bufs=3) as pbp,
                tc.tile_pool(name="cspool", bufs=2) as csp,
                tc.tile_pool(name="fin", bufs=2) as fin,
                tc.tile_pool(name="ps_s", bufs=3, space="PSUM") as ps_sp,
                tc.tile_pool(name="ps_u", bufs=2, space="PSUM") as ps_up,
                tc.tile_pool(name="ps_o", bufs=2, space="PSUM") as ps_op,
            ):
                for nb, (j, half) in enumerate(order):
                    # Scheduler-only fence (no runtime sync): keeps the tile
                    # scheduler from hoisting a later block's gather-dependent
                    # work ahead of this block in any engine stream, which
                    # would stall gather-free work on the collective when the
                    # cross-core launch skew is large.
                    tc.no_sync_barrier()
                    first, last = nb == 0, nb == NNB - 1
                    vtile = kvp.tile([P, VC * EMBED], BF16, tag="vtile")
                    if j == 0:
                        # own keys: K^T already in SBUF, V rows from x_shard
                        for c in range(VC):
                            nc.sync.dma_start(
                                out=vtile[:, c * EMBED : (c + 1) * EMBED],
                                in_=xs_r[half * VC + c],
                            )
                        k_off = half * NB

                        def k_slice(b):
                            return ktsb[:, b * M + k_off : b * M + k_off + NB]

                    else:
                        src = (rank + j) % NCORES
                        for c in range(VC):
                            nc.gpsimd.dma_start(
                                out=vtile[:, c * EMBED : (c + 1) * EMBED],
                                in_=xv_r[
                                    ds(src * (M // P) + half * VC + c, 1)
                                ].squeeze(0),
                            )
                        ktile = kvp.tile([P, DC * NB], BF16, tag="ktile")
                        for b in range(DC):
                            nc.gpsimd.dma_start(
                                out=ktile[:, b * NB : (b + 1) * NB],
                                in_=gkt_h[half][ds(src * DC + b, 1)].squeeze(0),
                            )

                        def k_slice(b, _kt=ktile):
                            return _kt[:, b * NB : (b + 1) * NB]

                    # scores + exp: P^T[key, query] in bf16
                    pt_sb = pbp.tile([P, VC * M], BF16, tag="pt_sb")
                    for h in range(M // NB):  # query column half
                        for c in range(VC):  # key chunk within block
                            ps_s = ps_sp.tile([P, NB], FP32, tag="ps_s")
                            for b in range(DC):
                                nc.tensor.matmul(
                                    ps_s[:],
                                    lhsT=k_slice(b)[:, c * P : (c + 1) * P],
                                    rhs=qt[:, b * M + h * NB : b * M + (h + 1) * NB],
                                    start=(b == 0),
                                    stop=(b == DC - 1),
                                )
                            nc.scalar.activation(
                                out=pt_sb[:, c * M + h * NB : c * M + (h + 1) * NB],
                                in_=ps_s[:],
                                func=EXP,
                            )
                    # key-chunk sums on Vector (emitted before any PV work so
                    # the adds for both halves sit early in the DVE queue)
                    cs = []
                    for h in range(M // NB):
                        cs01 = csp.tile([P, NB], BF16, tag="cs01")
                        cs23 = csp.tile([P, NB], BF16, tag="cs23")
                        csum = csp.tile([P, NB], BF16, tag="csum")
                        nc.vector.tensor_tensor(
                            out=cs01[:],
                            in0=pt_sb[:, 0 * M + h * NB : 0 * M + (h + 1) * NB],
                            in1=pt_sb[:, 1 * M + h * NB : 1 * M + (h + 1) * NB],
                            op=ADD,
                        )
                        nc.vector.tensor_tensor(
                            out=cs23[:],
                            in0=pt_sb[:, 2 * M + h * NB : 2 * M + (h + 1) * NB],
                            in1=pt_sb[:, 3 * M + h * NB : 3 * M + (h + 1) * NB],
                            op=ADD,
                        )
                        nc.vector.tensor_tensor(
                            out=csum[:], in0=cs01[:], in1=cs23[:], op=ADD
                        )
                        cs.append(csum)

                    def sums_pass(h):
                        # replicated partition-sum via ones-vector matmul
                        ps_sum = ps_up.tile([P, NB], FP32, tag="ps_sum")
                        nc.tensor.matmul(
                            ps_sum[:], lhsT=ones_sb[:], rhs=cs[h][:],
                            start=True, stop=True,
                        )
                        dsts = sums_acc[:, h * NB : (h + 1) * NB]
                        if first:
                            nc.vector.tensor_copy(out=dsts, in_=ps_sum[:])
                        else:
                            nc.vector.tensor_tensor(
                                out=dsts, in0=dsts, in1=ps_sum[:], op=ADD
                            )

                    def pv_pass(mb):
                        for h in range(EMBED // NB):
                            ps_o = ps_op.tile([P, NB], FP32, tag="ps_o")
                            for t in range(VC):
                                nc.tensor.matmul(
                                    ps_o[:],
                                    lhsT=pt_sb[:, t * M + mb * P : t * M + (mb + 1) * P],
                                    rhs=vtile[:, t * EMBED + h * NB : t * EMBED + (h + 1) * NB],
                                    start=(t == 0),
                                    stop=(t == VC - 1),
                                )
                            dst = out_acc[:, mb * EMBED + h * NB : mb * EMBED + (h + 1) * NB]
                            if first:
                                nc.vector.tensor_copy(out=dst, in_=ps_o[:])
                            else:
                                nc.vector.tensor_tensor(
                                    out=dst, in0=dst, in1=ps_o[:], op=ADD
                                )

                    def finalize(mb):
                        # divide by softmax sum, write out (overlaps later PV);
                        # on ScalarE (idle by now) to keep the DVE tail short
                        outf = fin.tile([P, EMBED], BF16, tag="outf")
                        nc.scalar.activation(
                            out=outf[:],
                            in_=out_acc[:, mb * EMBED : (mb + 1) * EMBED],
                            func=mybir.ActivationFunctionType.Copy,
                            scale=rtot[:, mb : mb + 1],
                        )
                        nc.sync.dma_start(out=out_r[mb], in_=outf[:])

                    # PE order keeps the sums matmuls behind enough PV work
                    # that their Vector-side inputs are always ready.
                    sums_pass(0)
                    pv_pass(0)
                    pv_pass(1)
                    sums_pass(1)
                    if last:
                        # sums_acc final: per-query reciprocal while PV of the
                        # remaining query blocks still runs on PE.
                        for mb in range(MB):
                            ps_f = ps_up.tile([P, P], FP32, tag="ps_f", bufs=1)
                            nc.tensor.transpose(
                                out=ps_f[:],
                                in_=sums_acc[:, mb * P : (mb + 1) * P],
                                identity=ident[:],
                            )
                            nc.vector.tensor_copy(
                                out=scol[:, mb : mb + 1], in_=ps_f[:, 0:1]
                            )
                        nc.vector.reciprocal(out=rtot[:], in_=scol[:])
                        finalize(0)
                        finalize(1)
                    for mb in range(2, MB):
                        pv_pass(mb)
                        if last:
                            finalize(mb)

    nc.compile()
    return nc


_NC = None


def _get_nc():
    global _NC
    if _NC is None:
        _NC = _build()
    return _NC


def _run(x, rotation_params, entangle_params, **spmd_kwargs):
    import ml_dtypes

    BF = ml_dtypes.bfloat16
    x = np.ascontiguousarray(np.asarray(x, dtype=np.float32))
    wq = np.asarray(rotation_params, dtype=np.float32).reshape(EMBED, EMBED) * np.float32(
        1.0 / 32.0
    )
    wk = np.asarray(entangle_params, dtype=np.float32).reshape(EMBED, EMBED)
    xt_bf = np.ascontiguousarray(x.T).astype(BF)
    x_bf = x.astype(BF)
    wq_bf = wq.astype(BF)
    wk_bf = wk.astype(BF)
    in_maps = [
        {
            "xt_shard": np.ascontiguousarray(xt_bf[:, i * M : (i + 1) * M]),
            "x_shard": np.ascontiguousarray(x_bf[i * M : (i + 1) * M]),
            "x_full": x_bf,
            "wq": wq_bf,
            "wk": wk_bf,
        }
        for i in range(NCORES)
    ]
    res = bass_utils.run_bass_kernel_spmd(
        _get_nc(), in_maps, core_ids=list(range(NCORES)), **spmd_kwargs
    )
    out = np.concatenate(
        [res.results[i]["out"].astype(np.float32) for i in range(NCORES)], axis=0
    )
    return out, res


def kernel(x, rotation_params, entangle_params):
    out, _ = _run(x, rotation_params, entangle_params)
    return out


# revision 6
# speedup vs baseline: 1.0423x; 1.0423x over previous
"""Trainium2 Bass kernel for ClassicalSelfAttention.

  out = softmax((x @ Wq) @ (x @ Wk)^T / sqrt(D)) @ x      x: [8192, 1024] f32

Sharding (8 NeuronCores): rows of x are sharded across cores; each core
projects its own row-shard to Q^T and K^T, the K^T shards are AllGathered
across cores in two halves along the key dim (each fired as soon as its
half of the K projection finishes, so the collective overlaps the rest of
the projection and the own-block compute), and each core runs a streaming
attention loop over 16 key-blocks of 512 keys: scores matmul -> exp on
ScalarE -> PV matmul accumulated in SBUF. The softmax division is folded
into the final output scale. 1/sqrt(1024) = 2^-5 is folded into Wq on the
host (exact). All matmuls run in bf16 with fp32 PSUM accumulation
(measured end-to-end rel err ~4e-3, well within tolerance); bf16 halves
the input DMA so the PE starts sooner.

The scores matmul keeps K^T stationary and Q^T moving, so PSUM holds
scores TRANSPOSED ([key, query]); exp of that is P^T directly -- exactly
the layout the PV matmul needs as its stationary operand -- so no PE
transposes are needed. The softmax row-sums (a partition-dim reduction in
this layout) are computed by first summing the four 128-key chunks on the
Vector engine and then a single ones-vector matmul per query half (2
matmuls per block instead of 8, saving ~25us of PE time); the per-block
emission order (scores, sums_h0, PV mb0, PV mb1, sums_h1, PV mb2..7)
keeps the PE from ever waiting on the Vector-engine chunk sums.

Block processing order: each core processes its OWN two key blocks first
straight out of SBUF (plus its own V rows from a per-core x_shard input),
then the half-0 key blocks of peers (rank+1..rank+7, via dynamic DMA
offsets), then the half-1 blocks; softmax over key blocks is
order-invariant so any order works. This gives the half-0 AllGather
~150us of compute cover and the half-1 AllGather ~380us.

The final division by the softmax sums is fused into the last key block:
the sums transposes/reciprocal run on PE/Vector between that block's PV
matmuls and each query-block's scaled output (bf16) DMAs out while the
PE is still working on later query blocks, removing the serial tail.
"""

import sys

import numpy as np

try:
    import concourse.bass as bass  # noqa: F401
except ImportError:  # pragma: no cover
    sys.path.insert(0, "/opt/trn_rl_repo")

import concourse.bacc as bacc
import concourse.mybir as mybir
import concourse.tile as tile
from concourse.masks import make_identity
from concourse import bass_utils
from concourse.bass import ds

N_TOKENS = 8192
EMBED = 1024
NCORES = 8
M = N_TOKENS // NCORES  # rows per core (1024)
P = 128  # partitions
DC = EMBED // P  # contraction chunks (8)
NB = 512  # key-block width
NNB = N_TOKENS // NB  # key blocks (16)
MB = M // P  # query row-blocks per core (8)
VC = NB // P  # value chunks per key block (4)
HPR = M // NB  # key-block halves per rank (2)
FP32 = mybir.dt.float32
BF16 = mybir.dt.bfloat16
EXP = mybir.ActivationFunctionType.Exp
ADD = mybir.AluOpType.add
AXX = mybir.AxisListType.X


def _build():
    nc = bacc.Bacc(
        "TRN2", target_bir_lowering=False, debug=False, num_devices=NCORES
    )
    xt_shard = nc.dram_tensor("xt_shard", [EMBED, M], BF16, kind="ExternalInput").ap()
    x_shard = nc.dram_tensor("x_shard", [M, EMBED], BF16, kind="ExternalInput").ap()
    x_full = nc.dram_tensor(
        "x_full", [N_TOKENS, EMBED], BF16, kind="ExternalInput"
    ).ap()
    wq_d = nc.dram_tensor("wq", [EMBED, EMBED], BF16, kind="ExternalInput").ap()
    wk_d = nc.dram_tensor("wk", [EMBED, EMBED], BF16, kind="ExternalInput").ap()
    out_d = nc.dram_tensor("out", [M, EMBED], BF16, kind="ExternalOutput").ap()

    wq_r = wq_d.rearrange("(a p) d -> a p d", p=P)  # [DC, P, EMBED]
    wk_r = wk_d.rearrange("(a p) d -> a p d", p=P)
    xt_r = xt_shard.rearrange("(a p) m -> a p m", p=P)  # [DC, P, M]
    xs_r = x_shard.rearrange("(t p) d -> t p d", p=P)  # [M//P, P, EMBED]
    xv_r = x_full.rearrange("(t p) d -> t p d", p=P)  # [64, P, EMBED]
    out_r = out_d.rearrange("(t p) d -> t p d", p=P)  # [MB, P, EMBED]

    with tile.TileContext(nc) as tc:
        with (
            tc.tile_pool(name="persist", bufs=1) as pers,
            tc.tile_pool(name="persist_dram", bufs=1, space="DRAM") as pdram,
        ):
            ones_sb = pers.tile([P, P], BF16)
            nc.vector.memset(ones_sb[:], 1.0)
            ident = pers.tile([P, P], FP32)
            make_identity(nc, ident[:])
            # Q^T resident for the whole kernel: qt[p, b*M + m] = Qt[b*128+p, m]
            qt = pers.tile([P, DC * M], BF16)
            # own K^T shard, kept resident: ktsb[p, b*M + n] = Kt_own[b*128+p, n]
            ktsb = pers.tile([P, DC * M], BF16)
            # fp32 PV accumulator per query block: [p, mb*EMBED + dv]
            out_acc = pers.tile([P, MB * EMBED], FP32)
            # softmax denominators, replicated across partitions: [p, m]
            sums_acc = pers.tile([P, M], FP32)
            # per-query reciprocal denominators: [p, mb] for query mb*128+p
            scol = pers.tile([P, MB], FP32)
            rtot = pers.tile([P, MB], FP32)
            # K^T shard halves (AllGather inputs) and gathered halves.
            # ktd_h[j][b] = own K^T rows b*128..(b+1)*128-1, keys half j.
            # gkt_h[j][r*DC + b] = same for rank r.
            ktd_h = [
                pdram.tile([DC, P, NB], BF16, name=f"ktd{j}", tag=f"ktd{j}")
                for j in range(HPR)
            ]
            gkt_h = [
                pdram.tile(
                    [NCORES * DC, P, NB],
                    BF16,
                    addr_space="Shared",
                    name=f"gkt{j}",
                    tag=f"gkt{j}",
                )
                for j in range(HPR)
            ]

            rank = nc.gpsimd.partition_id()

            # ---- Phase A: project K^T shard (AllGather each key half as it
            # completes), then Q^T (own rows).
            with (
                tc.tile_pool(name="proj", bufs=1) as proj,
                tc.tile_pool(name="proj_ps", bufs=4, space="PSUM") as proj_ps,
            ):
                wq_sb = proj.tile([P, DC * EMBED], BF16)
                wk_sb = proj.tile([P, DC * EMBED], BF16)
                xt_sb = proj.tile([P, DC * M], BF16)
                # DMA priority: wk + the j=0 key-half of xt (what the first
                # K-proj matmuls consume), then xt's j=1 half, then wq.
                for a in range(DC):
                    nc.sync.dma_start(
                        out=wk_sb[:, a * EMBED : (a + 1) * EMBED], in_=wk_r[a]
                    )
                    nc.sync.dma_start(
                        out=xt_sb[:, a * M : a * M + NB], in_=xt_r[a][:, 0:NB]
                    )
                for a in range(DC):
                    nc.sync.dma_start(
                        out=xt_sb[:, a * M + NB : (a + 1) * M], in_=xt_r[a][:, NB:M]
                    )
                for a in range(DC):
                    nc.sync.dma_start(
                        out=wq_sb[:, a * EMBED : (a + 1) * EMBED], in_=wq_r[a]
                    )
                # K^T first so its AllGathers overlap the Q^T projection;
                # key-half j outer so each half's gather fires at the
                # halfway point of the K projection.
                for j in range(HPR):
                    for b in range(DC):  # output dim chunk
                        ps = proj_ps.tile([P, NB], FP32, tag="proj_ps")
                        for a in range(DC):  # contraction chunk
                            nc.tensor.matmul(
                                ps[:],
                                lhsT=wk_sb[:, a * EMBED + b * P : a * EMBED + (b + 1) * P],
                                rhs=xt_sb[:, a * M + j * NB : a * M + (j + 1) * NB],
                                start=(a == 0),
                                stop=(a == DC - 1),
                            )
                        nc.vector.tensor_copy(
                            out=ktsb[:, b * M + j * NB : b * M + (j + 1) * NB],
                            in_=ps[:],
                        )
                        nc.sync.dma_start(
                            out=ktd_h[j][b],
                            in_=ktsb[:, b * M + j * NB : b * M + (j + 1) * NB],
                        )
                    nc.gpsimd.collective_compute(
                        "AllGather",
                        mybir.AluOpType.bypass,
                        replica_groups=[list(range(NCORES))],
                        ins=[ktd_h[j].opt()],
                        outs=[gkt_h[j].opt()],
                    )
                for b in range(DC):
                    for j in range(HPR):
                        ps = proj_ps.tile([P, NB], FP32, tag="proj_ps")
                        for a in range(DC):
                            nc.tensor.matmul(
                                ps[:],
                                lhsT=wq_sb[:, a * EMBED + b * P : a * EMBED + (b + 1) * P],
                                rhs=xt_sb[:, a * M + j * NB : a * M + (j + 1) * NB],
                                start=(a == 0),
                                stop=(a == DC - 1),
                            )
                        nc.vector.tensor_copy(
                            out=qt[:, b * M + j * NB : b * M + (j + 1) * NB],
                            in_=ps[:],
                        )

            # ---- Phase B: streaming attention over key blocks.
            # Own two blocks first (K^T already in SBUF), then peers'
            # half-0 blocks, then peers' half-1 blocks.
            order = [(0, h) for h in range(HPR)] + [
                (j, h) for h in range(HPR) for j in range(1, NCORES)
            ]
            with (
                tc.tile_pool(name="kv", bufs=2) as kvp,
                tc.tile_pool(name="pb", bufs=3) as pbp,
                tc.tile_pool(name="cspool", bufs=2) as csp,
                tc.tile_pool(name="fin", bufs=2) as fin,
                tc.tile_pool(name="ps_s", bufs=3, space="PSUM") as ps_sp,
                tc.tile_pool(name="ps_u", bufs=2, space="PSUM") as ps_up,
                tc.tile_pool(name="ps_o", bufs=2, space="PSUM") as ps_op,
            ):
                for nb, (j, half) in enumerate(order):
                    # Scheduler-only fence (no runtime sync): keeps the tile
                    # scheduler from hoisting a later block's gather-dependent
                    # work ahead of this block in any engine stream, which
                    # would stall gather-free work on the collective when the
                    # cross-core launch skew is large.
                    tc.no_sync_barrier()
                    first, last = nb == 0, nb == NNB - 1
                    vtile = kvp.tile([P, VC * EMBED], BF16, tag="vtile")
                    if j == 0:
                        # own keys: K^T already in SBUF, V rows from x_shard
                        for c in range(VC):
                            nc.sync.dma_start(
                                out=vtile[:, c * EMBED : (c + 1) * EMBED],
                                in_=xs_r[half * VC + c],
                            )
                        k_off = half * NB

                        def k_slice(b):
                            return ktsb[:, b * M + k_off : b * M + k_off + NB]

                    else:
                        src = (rank + j) % NCORES
                        for c in range(VC):
                            nc.gpsimd.dma_start(
                                out=vtile[:, c * EMBED : (c + 1) * EMBED],
                                in_=xv_r[
                                    ds(src * (M // P) + half * VC + c, 1)
                                ].squeeze(0),
                            )
                        ktile = kvp.tile([P, DC * NB], BF16, tag="ktile")
                        for b in range(DC):
                            nc.gpsimd.dma_start(
                                out=ktile[:, b * NB : (b + 1) * NB],
                                in_=gkt_h[half][ds(src * DC + b, 1)].squeeze(0),
                            )

                        def k_slice(b, _kt=ktile):
                            return _kt[:, b * NB : (b + 1) * NB]

                    # scores + exp: P^T[key, query] in bf16
                    pt_sb = pbp.tile([P, VC * M], BF16, tag="pt_sb")
                    for h in range(M // NB):  # query column half
                        for c in range(VC):  # key chunk within block
                            ps_s = ps_sp.tile([P, NB], FP32, tag="ps_s")
                            for b in range(DC):
                                nc.tensor.matmul(
                                    ps_s[:],
                                    lhsT=k_slice(b)[:, c * P : (c + 1) * P],
                                    rhs=qt[:, b * M + h * NB : b * M + (h + 1) * NB],
                                    start=(b == 0),
                                    stop=(b == DC - 1),
                                )
                            nc.scalar.activation(
                                out=pt_sb[:, c * M + h * NB : c * M + (h + 1) * NB],
                                in_=ps_s[:],
                                func=EXP,
                            )
                    # key-chunk sums on Vector (emitted before any PV work so
                    # the adds for both halves sit early in the DVE queue)
                    cs = []
                    for h in range(M // NB):
                        cs01 = csp.tile([P, NB], BF16, tag="cs01")
                        cs23 = csp.tile([P, NB], BF16, tag="cs23")
                        csum = csp.tile([P, NB], BF16, tag="csum")
                        nc.vector.tensor_tensor(
                            out=cs01[:],
                            in0=pt_sb[:, 0 * M + h * NB : 0 * M + (h + 1) * NB],
                            in1=pt_sb[:, 1 * M + h * NB : 1 * M + (h + 1) * NB],
                            op=ADD,
                        )
                        nc.vector.tensor_tensor(
                            out=cs23[:],
                            in0=pt_sb[:, 2 * M + h * NB : 2 * M + (h + 1) * NB],
                            in1=pt_sb[:, 3 * M + h * NB : 3 * M + (h + 1) * NB],
                            op=ADD,
                        )
                        nc.vector.tensor_tensor(
                            out=csum[:], in0=cs01[:], in1=cs23[:], op=ADD
                        )
                        cs.append(csum)

                    def sums_pass(h):
                        # replicated partition-sum via ones-vector matmul
                        ps_sum = ps_up.tile([P, NB], FP32, tag="ps_sum")
                        nc.tensor.matmul(
                            ps_sum[:], lhsT=ones_sb[:], rhs=cs[h][:],
                            start=True, stop=True,
                        )
                        dsts = sums_acc[:, h * NB : (h + 1) * NB]
                        if first:
                            nc.vector.tensor_copy(out=dsts, in_=ps_sum[:])
                        else:
                            nc.vector.tensor_tensor(
                                out=dsts, in0=dsts, in1=ps_sum[:], op=ADD
                            )

                    def pv_pass(mb):
                        for h in range(EMBED // NB):
                            ps_o = ps_op.tile([P, NB], FP32, tag="ps_o")
                            for t in range(VC):
                                nc.tensor.matmul(
                                    ps_o[:],
                                    lhsT=pt_sb[:, t * M + mb * P : t * M + (mb + 1) * P],
                                    rhs=vtile[:, t * EMBED + h * NB : t * EMBED + (h + 1) * NB],
                                    start=(t == 0),
                                    stop=(t == VC - 1),
                                )
                            dst = out_acc[:, mb * EMBED + h * NB : mb * EMBED + (h + 1) * NB]
                            if first:
                                nc.vector.tensor_copy(out=dst, in_=ps_o[:])
                            else:
                                nc.vector.tensor_tensor(
                                    out=dst, in0=dst, in1=ps_o[:], op=ADD
                                )

                    def finalize(mb):
                        # divide by softmax sum, write out (overlaps later PV);
                        # on ScalarE (idle by now) to keep the DVE tail short
                        outf = fin.tile([P, EMBED], BF16, tag="outf")
                        nc.scalar.activation(
                            out=outf[:],
                            in_=out_acc[:, mb * EMBED : (mb + 1) * EMBED],
                            func=mybir.ActivationFunctionType.Copy,
                            scale=rtot[:, mb : mb + 1],
                        )
                        nc.sync.dma_start(out=out_r[mb], in_=outf[:])

                    # PE order keeps the sums matmuls behind enough PV work
                    # that their Vector-side inputs are always ready.
                    sums_pass(0)
                    pv_pass(0)
                    pv_pass(1)
                    sums_pass(1)
                    if last:
                        # sums_acc final: per-query reciprocal while PV of the
                        # remaining query blocks still runs on PE.
                        for mb in range(MB):
                            ps_f = ps_up.tile([P, P], FP32, tag="ps_f", bufs=1)
                            nc.tensor.transpose(
                                out=ps_f[:],
                                in_=sums_acc[:, mb * P : (mb + 1) * P],
                                identity=ident[:],
                            )
                            nc.vector.tensor_copy(
                                out=scol[:, mb : mb + 1], in_=ps_f[:, 0:1]
                            )
                        nc.vector.reciprocal(out=rtot[:], in_=scol[:])
                        finalize(0)
                        finalize(1)
                    for mb in range(2, MB):
                        pv_pass(mb)
                        if last:
                            finalize(mb)

    nc.compile()
    return nc


_NC = None


def _get_nc():
    global _NC
    if _NC is None:
        _NC = _build()
    return _NC


def _run(x, rotation_params, entangle_params, **spmd_kwargs):
    import ml_dtypes

    BF = ml_dtypes.bfloat16
    x = np.ascontiguousarray(np.asarray(x, dtype=np.float32))
    wq = np.asarray(rotation_params, dtype=np.float32).reshape(EMBED, EMBED) * np.float32(
        1.0 / 32.0
    )
    wk = np.asarray(entangle_params, dtype=np.float32).reshape(EMBED, EMBED)
    xt_bf = np.ascontiguousarray(x.T).astype(BF)
    x_bf = x.astype(BF)
    wq_bf = wq.astype(BF)
    wk_bf = wk.astype(BF)
    in_maps = [
        {
            "xt_shard": np.ascontiguousarray(xt_bf[:, i * M : (i + 1) * M]),
            "x_shard": np.ascontiguousarray(x_bf[i * M : (i + 1) * M]),
            "x_full": x_bf,
            "wq": wq_bf,
            "wk": wk_bf,
        }
        for i in range(NCORES)
    ]
    res = bass_utils.run_bass_kernel_spmd(
        _get_nc(), in_maps, core_ids=list(range(NCORES)), **spmd_kwargs
    )
    out = np.concatenate(
        [res.results[i]["out"].astype(np.float32) for i in range(NCORES)], axis=0
    )
    return out, res


def kernel(x, rotation_params, entangle_params):
    out, _ = _run(x, rotation_params, entangle_params)
    return out


# revision 7
# speedup vs baseline: 1.0879x; 1.0437x over previous
"""Trainium2 Bass kernel for ClassicalSelfAttention.

  out = softmax((x @ Wq) @ (x @ Wk)^T / sqrt(D)) @ x      x: [8192, 1024] f32

Sharding (8 NeuronCores): rows of x are sharded across cores; each core
projects its own row-shard to Q^T and K^T, the K^T shards are AllGathered
across cores in two halves along the key dim (each fired as soon as its
half of the K projection finishes, so the collective overlaps the rest of
the projection and the own-block compute), and each core runs a streaming
attention loop over 16 key-blocks of 512 keys: scores matmul -> exp on
ScalarE -> PV matmul accumulated in SBUF. The softmax division is folded
into the final output scale. 1/sqrt(1024) = 2^-5 is folded into Wq on the
host (exact). All matmuls run in bf16 with fp32 PSUM accumulation
(measured end-to-end rel err ~4e-3, well within tolerance); bf16 halves
the input DMA so the PE starts sooner.

The scores matmul keeps K^T stationary and Q^T moving, so PSUM holds
scores TRANSPOSED ([key, query]); exp of that is P^T directly -- exactly
the layout the PV matmul needs as its stationary operand -- so no PE
transposes are needed. The softmax row-sums (a partition-dim reduction in
this layout) are computed by first summing the four 128-key chunks on the
Vector engine and then a single ones-vector matmul per query half (2
matmuls per block instead of 8, saving ~25us of PE time); the per-block
emission order (scores, sums_h0, PV mb0, PV mb1, sums_h1, PV mb2..7)
keeps the PE from ever waiting on the Vector-engine chunk sums.

Block processing order: each core processes its OWN two key blocks first
straight out of SBUF (plus its own V rows from a per-core x_shard input),
then the half-0 key blocks of peers (rank+1..rank+7, via dynamic DMA
offsets), then the half-1 blocks; softmax over key blocks is
order-invariant so any order works. This gives the half-0 AllGather
~150us of compute cover and the half-1 AllGather ~380us.

The final division by the softmax sums is fused into the last key block:
the sums transposes/reciprocal run on PE/Vector between that block's PV
matmuls and each query-block's scaled output (bf16) DMAs out while the
PE is still working on later query blocks, removing the serial tail.
"""

import sys

import numpy as np

try:
    import concourse.bass as bass  # noqa: F401
except ImportError:  # pragma: no cover
    sys.path.insert(0, "/opt/trn_rl_repo")

import concourse.bacc as bacc
import concourse.mybir as mybir
import concourse.tile as tile
from concourse.masks import make_identity
from concourse import bass_utils
from concourse.bass import ds

N_TOKENS = 8192
EMBED = 1024
NCORES = 8
M = N_TOKENS // NCORES  # rows per core (1024)
P = 128  # partitions
DC = EMBED // P  # contraction chunks (8)
NB = 512  # key-block width
NNB = N_TOKENS // NB  # key blocks (16)
MB = M // P  # query row-blocks per core (8)
VC = NB // P  # value chunks per key block (4)
HPR = M // NB  # key-block halves per rank (2)
FP32 = mybir.dt.float32
BF16 = mybir.dt.bfloat16
EXP = mybir.ActivationFunctionType.Exp
ADD = mybir.AluOpType.add
AXX = mybir.AxisListType.X


def _build():
    nc = bacc.Bacc(
        "TRN2", target_bir_lowering=False, debug=False, num_devices=NCORES
    )
    xt_shard = nc.dram_tensor("xt_shard", [EMBED, M], BF16, kind="ExternalInput").ap()
    x_shard = nc.dram_tensor("x_shard", [M, EMBED], BF16, kind="ExternalInput").ap()
    x_full = nc.dram_tensor(
        "x_full", [N_TOKENS, EMBED], BF16, kind="ExternalInput"
    ).ap()
    wq_d = nc.dram_tensor("wq", [EMBED, EMBED], BF16, kind="ExternalInput").ap()
    wk_d = nc.dram_tensor("wk", [EMBED, EMBED], BF16, kind="ExternalInput").ap()
    out_d = nc.dram_tensor("out", [M, EMBED], BF16, kind="ExternalOutput").ap()

    wq_r = wq_d.rearrange("(a p) d -> a p d", p=P)  # [DC, P, EMBED]
    wk_r = wk_d.rearrange("(a p) d -> a p d", p=P)
    xt_r = xt_shard.rearrange("(a p) m -> a p m", p=P)  # [DC, P, M]
    xs_r = x_shard.rearrange("(t p) d -> t p d", p=P)  # [M//P, P, EMBED]
    xv_r = x_full.rearrange("(t p) d -> t p d", p=P)  # [64, P, EMBED]
    out_r = out_d.rearrange("(t p) d -> t p d", p=P)  # [MB, P, EMBED]

    with tile.TileContext(nc) as tc:
        with (
            tc.tile_pool(name="persist", bufs=1) as pers,
            tc.tile_pool(name="persist_dram", bufs=1, space="DRAM") as pdram,
        ):
            ones_sb = pers.tile([P, P], BF16)
            nc.vector.memset(ones_sb[:], 1.0)
            ident = pers.tile([P, P], FP32)
            make_identity(nc, ident[:])
            # Q^T resident for the whole kernel: qt[p, b*M + m] = Qt[b*128+p, m]
            qt = pers.tile([P, DC * M], BF16)
            # own K^T shard, kept resident: ktsb[p, b*M + n] = Kt_own[b*128+p, n]
            ktsb = pers.tile([P, DC * M], BF16)
            # fp32 PV accumulator per query block: [p, mb*EMBED + dv]
            out_acc = pers.tile([P, MB * EMBED], FP32)
            # softmax denominators, replicated across partitions: [p, m]
            sums_acc = pers.tile([P, M], FP32)
            # per-query reciprocal denominators: [p, mb] for query mb*128+p
            scol = pers.tile([P, MB], FP32)
            rtot = pers.tile([P, MB], FP32)
            # K^T shard halves (AllGather inputs) and gathered halves.
            # ktd_h[j][b] = own K^T rows b*128..(b+1)*128-1, keys half j.
            # gkt_h[j][r*DC + b] = same for rank r.
            ktd_h = [
                pdram.tile([DC, P, NB], BF16, name=f"ktd{j}", tag=f"ktd{j}")
                for j in range(HPR)
            ]
            gkt_h = [
                pdram.tile(
                    [NCORES * DC, P, NB],
                    BF16,
                    addr_space="Shared",
                    name=f"gkt{j}",
                    tag=f"gkt{j}",
                )
                for j in range(HPR)
            ]

            rank = nc.gpsimd.partition_id()

            # ---- Phase A: project K^T shard (AllGather each key half as it
            # completes), then Q^T (own rows).
            with (
                tc.tile_pool(name="proj", bufs=1) as proj,
                tc.tile_pool(name="proj_ps", bufs=4, space="PSUM") as proj_ps,
            ):
                wq_sb = proj.tile([P, DC * EMBED], BF16)
                wk_sb = proj.tile([P, DC * EMBED], BF16)
                xt_sb = proj.tile([P, DC * M], BF16)
                # DMA priority: wk + the j=0 key-half of xt (what the first
                # K-proj matmuls consume), then xt's j=1 half, then wq.
                for a in range(DC):
                    nc.sync.dma_start(
                        out=wk_sb[:, a * EMBED : (a + 1) * EMBED], in_=wk_r[a]
                    )
                    nc.sync.dma_start(
                        out=xt_sb[:, a * M : a * M + NB], in_=xt_r[a][:, 0:NB]
                    )
                for a in range(DC):
                    nc.sync.dma_start(
                        out=xt_sb[:, a * M + NB : (a + 1) * M], in_=xt_r[a][:, NB:M]
                    )
                for a in range(DC):
                    nc.sync.dma_start(
                        out=wq_sb[:, a * EMBED : (a + 1) * EMBED], in_=wq_r[a]
                    )
                # K^T first so its AllGathers overlap the Q^T projection;
                # key-half j outer so each half's gather fires at the
                # halfway point of the K projection.
                for j in range(HPR):
                    for bp in range(0, DC, 2):  # output dim chunk pairs
                        pss = {}
                        if j == 0:
                            # split contraction (a<4 | a>=4) so the first
                            # matmuls only need half the wk/xt DMA landed
                            for half_a, tag in ((0, "pp1"), (1, "pp2")):
                                for b in (bp, bp + 1):
                                    ps = proj_ps.tile(
                                        [P, NB], FP32, tag=tag, bufs=2
                                    )
                                    if half_a == 0:
                                        pss[b] = [ps]
                                    else:
                                        pss[b].append(ps)
                                    for a in range(
                                        half_a * (DC // 2),
                                        (half_a + 1) * (DC // 2),
                                    ):
                                        nc.tensor.matmul(
                                            ps[:],
                                            lhsT=wk_sb[:, a * EMBED + b * P : a * EMBED + (b + 1) * P],
                                            rhs=xt_sb[:, a * M + j * NB : a * M + (j + 1) * NB],
                                            start=(a % (DC // 2) == 0),
                                            stop=(a % (DC // 2) == DC // 2 - 1),
                                        )
                        else:
                            for b in (bp, bp + 1):
                                ps = proj_ps.tile([P, NB], FP32, tag="proj_ps")
                                pss[b] = [ps]
                                for a in range(DC):  # contraction chunk
                                    nc.tensor.matmul(
                                        ps[:],
                                        lhsT=wk_sb[:, a * EMBED + b * P : a * EMBED + (b + 1) * P],
                                        rhs=xt_sb[:, a * M + j * NB : a * M + (j + 1) * NB],
                                        start=(a == 0),
                                        stop=(a == DC - 1),
                                    )
                        for b in (bp, bp + 1):
                            dst = ktsb[:, b * M + j * NB : b * M + (j + 1) * NB]
                            nc.vector.tensor_copy(out=dst, in_=pss[b][0][:])
                            if len(pss[b]) > 1:
                                nc.vector.tensor_tensor(
                                    out=dst, in0=dst, in1=pss[b][1][:], op=ADD
                                )
                            nc.sync.dma_start(out=ktd_h[j][b], in_=dst)
                    nc.gpsimd.collective_compute(
                        "AllGather",
                        mybir.AluOpType.bypass,
                        replica_groups=[list(range(NCORES))],
                        ins=[ktd_h[j].opt()],
                        outs=[gkt_h[j].opt()],
                    )
                for b in range(DC):
                    for j in range(HPR):
                        ps = proj_ps.tile([P, NB], FP32, tag="proj_ps")
                        for a in range(DC):
                            nc.tensor.matmul(
                                ps[:],
                                lhsT=wq_sb[:, a * EMBED + b * P : a * EMBED + (b + 1) * P],
                                rhs=xt_sb[:, a * M + j * NB : a * M + (j + 1) * NB],
                                start=(a == 0),
                                stop=(a == DC - 1),
                            )
                        nc.vector.tensor_copy(
                            out=qt[:, b * M + j * NB : b * M + (j + 1) * NB],
                            in_=ps[:],
                        )

            # ---- Phase B: streaming attention over key blocks.
            # Own two blocks first (K^T already in SBUF), then peers'
            # half-0 blocks, then peers' half-1 blocks.
            order = [(0, h) for h in range(HPR)] + [
                (j, h) for h in range(HPR) for j in range(1, NCORES)
            ]
            with (
                tc.tile_pool(name="kv", bufs=2) as kvp,
                tc.tile_pool(name="pb", bufs=3) as pbp,
                tc.tile_pool(name="cspool", bufs=2) as csp,
                tc.tile_pool(name="fin", bufs=2) as fin,
                tc.tile_pool(name="ps_s", bufs=3, space="PSUM") as ps_sp,
                tc.tile_pool(name="ps_u", bufs=2, space="PSUM") as ps_up,
                tc.tile_pool(name="ps_o", bufs=2, space="PSUM") as ps_op,
            ):
                for nb, (j, half) in enumerate(order):
                    # Scheduler-only fence (no runtime sync): keeps the tile
                    # scheduler from hoisting a later block's gather-dependent
                    # work ahead of this block in any engine stream, which
                    # would stall gather-free work on the collective when the
                    # cross-core launch skew is large.
                    tc.no_sync_barrier()
                    first, last = nb == 0, nb == NNB - 1
                    vtile = kvp.tile([P, VC * EMBED], BF16, tag="vtile")
                    if j == 0:
                        # own keys: K^T already in SBUF, V rows from x_shard
                        for c in range(VC):
                            nc.sync.dma_start(
                                out=vtile[:, c * EMBED : (c + 1) * EMBED],
                                in_=xs_r[half * VC + c],
                            )
                        k_off = half * NB

                        def k_slice(b):
                            return ktsb[:, b * M + k_off : b * M + k_off + NB]

                    else:
                        src = (rank + j) % NCORES
                        for c in range(VC):
                            nc.gpsimd.dma_start(
                                out=vtile[:, c * EMBED : (c + 1) * EMBED],
                                in_=xv_r[
                                    ds(src * (M // P) + half * VC + c, 1)
                                ].squeeze(0),
                            )
                        ktile = kvp.tile([P, DC * NB], BF16, tag="ktile")
                        for b in range(DC):
                            nc.gpsimd.dma_start(
                                out=ktile[:, b * NB : (b + 1) * NB],
                                in_=gkt_h[half][ds(src * DC + b, 1)].squeeze(0),
                            )

                        def k_slice(b, _kt=ktile):
                            return _kt[:, b * NB : (b + 1) * NB]

                    # scores + exp: P^T[key, query] in bf16
                    pt_sb = pbp.tile([P, VC * M], BF16, tag="pt_sb")
                    for h in range(M // NB):  # query column half
                        for c in range(VC):  # key chunk within block
                            ps_s = ps_sp.tile([P, NB], FP32, tag="ps_s")
                            for b in range(DC):
                                nc.tensor.matmul(
                                    ps_s[:],
                                    lhsT=k_slice(b)[:, c * P : (c + 1) * P],
                                    rhs=qt[:, b * M + h * NB : b * M + (h + 1) * NB],
                                    start=(b == 0),
                                    stop=(b == DC - 1),
                                )
                            nc.scalar.activation(
                                out=pt_sb[:, c * M + h * NB : c * M + (h + 1) * NB],
                                in_=ps_s[:],
                                func=EXP,
                            )
                    # key-chunk sums on Vector (emitted before any PV work so
                    # the adds for both halves sit early in the DVE queue)
                    cs = []
                    for h in range(M // NB):
                        cs01 = csp.tile([P, NB], BF16, tag="cs01")
                        cs23 = csp.tile([P, NB], BF16, tag="cs23")
                        csum = csp.tile([P, NB], BF16, tag="csum")
                        nc.vector.tensor_tensor(
                            out=cs01[:],
                            in0=pt_sb[:, 0 * M + h * NB : 0 * M + (h + 1) * NB],
                            in1=pt_sb[:, 1 * M + h * NB : 1 * M + (h + 1) * NB],
                            op=ADD,
                        )
                        nc.vector.tensor_tensor(
                            out=cs23[:],
                            in0=pt_sb[:, 2 * M + h * NB : 2 * M + (h + 1) * NB],
                            in1=pt_sb[:, 3 * M + h * NB : 3 * M + (h + 1) * NB],
                            op=ADD,
                        )
                        nc.vector.tensor_tensor(
                            out=csum[:], in0=cs01[:], in1=cs23[:], op=ADD
                        )
                        cs.append(csum)

                    def sums_pass(h):
                        # replicated partition-sum via ones-vector matmul
                        ps_sum = ps_up.tile([P, NB], FP32, tag="ps_sum")
                        nc.tensor.matmul(
                            ps_sum[:], lhsT=ones_sb[:], rhs=cs[h][:],
                            start=True, stop=True,
                        )
                        dsts = sums_acc[:, h * NB : (h + 1) * NB]
                        if first:
                            nc.vector.tensor_copy(out=dsts, in_=ps_sum[:])
                        else:
                            nc.vector.tensor_tensor(
                                out=dsts, in0=dsts, in1=ps_sum[:], op=ADD
                            )

                    def pv_pass(mb):
                        for h in range(EMBED // NB):
                            ps_o = ps_op.tile([P, NB], FP32, tag="ps_o")
                            for t in range(VC):
                                nc.tensor.matmul(
                                    ps_o[:],
                                    lhsT=pt_sb[:, t * M + mb * P : t * M + (mb + 1) * P],
                                    rhs=vtile[:, t * EMBED + h * NB : t * EMBED + (h + 1) * NB],
                                    start=(t == 0),
                                    stop=(t == VC - 1),
                                )
                            dst = out_acc[:, mb * EMBED + h * NB : mb * EMBED + (h + 1) * NB]
                            if first:
                                nc.vector.tensor_copy(out=dst, in_=ps_o[:])
                            else:
                                nc.vector.tensor_tensor(
                                    out=dst, in0=dst, in1=ps_o[:], op=ADD
                                )

                    def finalize(mb):
                        # divide by softmax sum, write out (overlaps later PV);
                        # on ScalarE (idle by now) to keep the DVE tail short
                        outf = fin.tile([P, EMBED], BF16, tag="outf")
                        nc.scalar.activation(
                            out=outf[:],
                            in_=out_acc[:, mb * EMBED : (mb + 1) * EMBED],
                            func=mybir.ActivationFunctionType.Copy,
                            scale=rtot[:, mb : mb + 1],
                        )
                        nc.sync.dma_start(out=out_r[mb], in_=outf[:])

                    # PE order keeps the sums matmuls behind enough PV work
                    # that their Vector-side inputs are always ready.
                    sums_pass(0)
                    pv_pass(0)
                    pv_pass(1)
                    sums_pass(1)
                    if last:
                        # sums_acc final: per-query reciprocal while PV of the
                        # remaining query blocks still runs on PE.
                        for mb in range(MB):
                            ps_f = ps_up.tile([P, P], FP32, tag="ps_f", bufs=1)
                            nc.tensor.transpose(
                                out=ps_f[:],
                                in_=sums_acc[:, mb * P : (mb + 1) * P],
                                identity=ident[:],
                            )
                            nc.vector.tensor_copy(
                                out=scol[:, mb : mb + 1], in_=ps_f[:, 0:1]
                            )
                        nc.vector.reciprocal(out=rtot[:], in_=scol[:])
                        finalize(0)
                        finalize(1)
                    for mb in range(2, MB):
                        pv_pass(mb)
                        if last:
                            finalize(mb)

    nc.compile()
    return nc


_NC = None


def _get_nc():
    global _NC
    if _NC is None:
        _NC = _build()
    return _NC


def _run(x, rotation_params, entangle_params, **spmd_kwargs):
    import ml_dtypes

    BF = ml_dtypes.bfloat16
    x = np.ascontiguousarray(np.asarray(x, dtype=np.float32))
    wq = np.asarray(rotation_params, dtype=np.float32).reshape(EMBED, EMBED) * np.float32(
        1.0 / 32.0
    )
    wk = np.asarray(entangle_params, dtype=np.float32).reshape(EMBED, EMBED)
    xt_bf = np.ascontiguousarray(x.T).astype(BF)
    x_bf = x.astype(BF)
    wq_bf = wq.astype(BF)
    wk_bf = wk.astype(BF)
    in_maps = [
        {
            "xt_shard": np.ascontiguousarray(xt_bf[:, i * M : (i + 1) * M]),
            "x_shard": np.ascontiguousarray(x_bf[i * M : (i + 1) * M]),
            "x_full": x_bf,
            "wq": wq_bf,
            "wk": wk_bf,
        }
        for i in range(NCORES)
    ]
    res = bass_utils.run_bass_kernel_spmd(
        _get_nc(), in_maps, core_ids=list(range(NCORES)), **spmd_kwargs
    )
    out = np.concatenate(
        [res.results[i]["out"].astype(np.float32) for i in range(NCORES)], axis=0
    )
    return out, res


def kernel(x, rotation_params, entangle_params):
    out, _ = _run(x, rotation_params, entangle_params)
    return out


# revision 8
# speedup vs baseline: 1.0895x; 1.0015x over previous
"""Trainium2 Bass kernel for ClassicalSelfAttention.

  out = softmax((x @ Wq) @ (x @ Wk)^T / sqrt(D)) @ x      x: [8192, 1024] f32

Sharding (8 NeuronCores): rows of x are sharded across cores; each core
projects its own row-shard to Q^T and K^T, the K^T shards are AllGathered
across cores in two halves along the key dim (each fired as soon as its
half of the K projection finishes, so the collective overlaps the rest of
the projection and the own-block compute), and each core runs a streaming
attention loop over 16 key-blocks of 512 keys: scores matmul -> exp on
ScalarE -> PV matmul accumulated in SBUF. The softmax division is folded
into the final output scale. 1/sqrt(1024) = 2^-5 is folded into Wq on the
host (exact). All matmuls run in bf16 with fp32 PSUM accumulation
(measured end-to-end rel err ~4e-3, well within tolerance); bf16 halves
the input DMA so the PE starts sooner.

The scores matmul keeps K^T stationary and Q^T moving, so PSUM holds
scores TRANSPOSED ([key, query]); exp of that is P^T directly -- exactly
the layout the PV matmul needs as its stationary operand -- so no PE
transposes are needed. The softmax row-sums (a partition-dim reduction in
this layout) are computed by first summing the four 128-key chunks on the
Vector engine and then a single ones-vector matmul per query half (2
matmuls per block instead of 8, saving ~25us of PE time); the per-block
emission order (scores, sums_h0, PV mb0, PV mb1, sums_h1, PV mb2..7)
keeps the PE from ever waiting on the Vector-engine chunk sums.

Block processing order: each core processes its OWN two key blocks first
straight out of SBUF (plus its own V rows from a per-core x_shard input),
then the half-0 key blocks of peers (rank+1..rank+7, via dynamic DMA
offsets), then the half-1 blocks; softmax over key blocks is
order-invariant so any order works. This gives the half-0 AllGather
~150us of compute cover and the half-1 AllGather ~380us.

The final division by the softmax sums is fused into the last key block:
the sums transposes/reciprocal run on PE/Vector between that block's PV
matmuls and each query-block's scaled output (bf16) DMAs out while the
PE is still working on later query blocks, removing the serial tail.
"""

import sys

import numpy as np

try:
    import concourse.bass as bass  # noqa: F401
except ImportError:  # pragma: no cover
    sys.path.insert(0, "/opt/trn_rl_repo")

import concourse.bacc as bacc
import concourse.mybir as mybir
import concourse.tile as tile
from concourse.masks import make_identity
from concourse import bass_utils
from concourse.bass import ds

N_TOKENS = 8192
EMBED = 1024
NCORES = 8
M = N_TOKENS // NCORES  # rows per core (1024)
P = 128  # partitions
DC = EMBED // P  # contraction chunks (8)
NB = 512  # key-block width
NNB = N_TOKENS // NB  # key blocks (16)
MB = M // P  # query row-blocks per core (8)
VC = NB // P  # value chunks per key block (4)
HPR = M // NB  # key-block halves per rank (2)
FP32 = mybir.dt.float32
BF16 = mybir.dt.bfloat16
EXP = mybir.ActivationFunctionType.Exp
ADD = mybir.AluOpType.add
AXX = mybir.AxisListType.X


def _build():
    nc = bacc.Bacc(
        "TRN2", target_bir_lowering=False, debug=False, num_devices=NCORES
    )
    xt_shard = nc.dram_tensor("xt_shard", [EMBED, M], BF16, kind="ExternalInput").ap()
    x_shard = nc.dram_tensor("x_shard", [M, EMBED], BF16, kind="ExternalInput").ap()
    x_full = nc.dram_tensor(
        "x_full", [N_TOKENS, EMBED], BF16, kind="ExternalInput"
    ).ap()
    wq_d = nc.dram_tensor("wq", [EMBED, EMBED], BF16, kind="ExternalInput").ap()
    wk_d = nc.dram_tensor("wk", [EMBED, EMBED], BF16, kind="ExternalInput").ap()
    out_d = nc.dram_tensor("out", [M, EMBED], BF16, kind="ExternalOutput").ap()

    wq_r = wq_d.rearrange("(a p) d -> a p d", p=P)  # [DC, P, EMBED]
    wk_r = wk_d.rearrange("(a p) d -> a p d", p=P)
    xt_r = xt_shard.rearrange("(a p) m -> a p m", p=P)  # [DC, P, M]
    xs_r = x_shard.rearrange("(t p) d -> t p d", p=P)  # [M//P, P, EMBED]
    xv_r = x_full.rearrange("(t p) d -> t p d", p=P)  # [64, P, EMBED]
    out_r = out_d.rearrange("(t p) d -> t p d", p=P)  # [MB, P, EMBED]

    with tile.TileContext(nc) as tc:
        with (
            tc.tile_pool(name="persist", bufs=1) as pers,
            tc.tile_pool(name="persist_dram", bufs=1, space="DRAM") as pdram,
        ):
            ones_sb = pers.tile([P, P], BF16)
            nc.vector.memset(ones_sb[:], 1.0)
            ident = pers.tile([P, P], FP32)
            make_identity(nc, ident[:])
            # Q^T resident for the whole kernel: qt[p, b*M + m] = Qt[b*128+p, m]
            qt = pers.tile([P, DC * M], BF16)
            # own K^T shard, kept resident: ktsb[p, b*M + n] = Kt_own[b*128+p, n]
            ktsb = pers.tile([P, DC * M], BF16)
            # fp32 PV accumulator per query block: [p, mb*EMBED + dv]
            out_acc = pers.tile([P, MB * EMBED], FP32)
            # softmax denominators, replicated across partitions: [p, m]
            sums_acc = pers.tile([P, M], FP32)
            # per-query reciprocal denominators: [p, mb] for query mb*128+p
            scol = pers.tile([P, MB], FP32)
            rtot = pers.tile([P, MB], FP32)
            # K^T shard halves (AllGather inputs) and gathered halves.
            # ktd_h[j][b] = own K^T rows b*128..(b+1)*128-1, keys half j.
            # gkt_h[j][r*DC + b] = same for rank r.
            ktd_h = [
                pdram.tile([DC, P, NB], BF16, name=f"ktd{j}", tag=f"ktd{j}")
                for j in range(HPR)
            ]
            gkt_h = [
                pdram.tile(
                    [NCORES * DC, P, NB],
                    BF16,
                    addr_space="Shared",
                    name=f"gkt{j}",
                    tag=f"gkt{j}",
                )
                for j in range(HPR)
            ]

            rank = nc.gpsimd.partition_id()

            # ---- Phase A: project K^T shard (AllGather each key half as it
            # completes), then Q^T (own rows).
            with (
                tc.tile_pool(name="proj", bufs=1) as proj,
                tc.tile_pool(name="proj_ps", bufs=4, space="PSUM") as proj_ps,
            ):
                wq_sb = proj.tile([P, DC * EMBED], BF16)
                wk_sb = proj.tile([P, DC * EMBED], BF16)
                xt_sb = proj.tile([P, DC * M], BF16)
                # DMA priority: wk + the j=0 key-half of xt (what the first
                # K-proj matmuls consume), then xt's j=1 half, then wq.
                for a in range(DC):
                    nc.sync.dma_start(
                        out=wk_sb[:, a * EMBED : (a + 1) * EMBED], in_=wk_r[a]
                    )
                    nc.sync.dma_start(
                        out=xt_sb[:, a * M : a * M + NB], in_=xt_r[a][:, 0:NB]
                    )
                for a in range(DC):
                    nc.sync.dma_start(
                        out=xt_sb[:, a * M + NB : (a + 1) * M], in_=xt_r[a][:, NB:M]
                    )
                for a in range(DC):
                    nc.sync.dma_start(
                        out=wq_sb[:, a * EMBED : (a + 1) * EMBED], in_=wq_r[a]
                    )
                # K^T first so its AllGathers overlap the Q^T projection;
                # key-half j outer so each half's gather fires at the
                # halfway point of the K projection.
                for j in range(HPR):
                    for b in range(DC):  # output dim chunk
                        ps = proj_ps.tile([P, NB], FP32, tag="proj_ps")
                        for a in range(DC):  # contraction chunk
                            nc.tensor.matmul(
                                ps[:],
                                lhsT=wk_sb[:, a * EMBED + b * P : a * EMBED + (b + 1) * P],
                                rhs=xt_sb[:, a * M + j * NB : a * M + (j + 1) * NB],
                                start=(a == 0),
                                stop=(a == DC - 1),
                            )
                        nc.vector.tensor_copy(
                            out=ktsb[:, b * M + j * NB : b * M + (j + 1) * NB],
                            in_=ps[:],
                        )
                        nc.sync.dma_start(
                            out=ktd_h[j][b],
                            in_=ktsb[:, b * M + j * NB : b * M + (j + 1) * NB],
                        )
                    nc.gpsimd.collective_compute(
                        "AllGather",
                        mybir.AluOpType.bypass,
                        replica_groups=[list(range(NCORES))],
                        ins=[ktd_h[j].opt()],
                        outs=[gkt_h[j].opt()],
                    )
                for b in range(DC):
                    for j in range(HPR):
                        ps = proj_ps.tile([P, NB], FP32, tag="proj_ps")
                        for a in range(DC):
                            nc.tensor.matmul(
                                ps[:],
                                lhsT=wq_sb[:, a * EMBED + b * P : a * EMBED + (b + 1) * P],
                                rhs=xt_sb[:, a * M + j * NB : a * M + (j + 1) * NB],
                                start=(a == 0),
                                stop=(a == DC - 1),
                            )
                        nc.vector.tensor_copy(
                            out=qt[:, b * M + j * NB : b * M + (j + 1) * NB],
                            in_=ps[:],
                        )

            # ---- Phase B: streaming attention over key blocks.
            # Own two blocks first (K^T already in SBUF), then peers'
            # half-0 blocks, then peers' half-1 blocks.
            order = [(0, h) for h in range(HPR)] + [
                (j, h) for h in range(HPR) for j in range(1, NCORES)
            ]
            with (
                tc.tile_pool(name="kv", bufs=2) as kvp,
                tc.tile_pool(name="pb", bufs=3) as pbp,
                tc.tile_pool(name="cspool", bufs=2) as csp,
                tc.tile_pool(name="fin", bufs=2) as fin,
                tc.tile_pool(name="ps_s", bufs=3, space="PSUM") as ps_sp,
                tc.tile_pool(name="ps_u", bufs=2, space="PSUM") as ps_up,
                tc.tile_pool(name="ps_o", bufs=2, space="PSUM") as ps_op,
            ):
                for nb, (j, half) in enumerate(order):
                    # Scheduler-only fence (no runtime sync): keeps the tile
                    # scheduler from hoisting a later block's gather-dependent
                    # work ahead of this block in any engine stream, which
                    # would stall gather-free work on the collective when the
                    # cross-core launch skew is large.
                    tc.no_sync_barrier()
                    first, last = nb == 0, nb == NNB - 1
                    vtile = kvp.tile([P, VC * EMBED], BF16, tag="vtile")
                    if j == 0:
                        # own keys: K^T already in SBUF, V rows from x_shard
                        for c in range(VC):
                            nc.sync.dma_start(
                                out=vtile[:, c * EMBED : (c + 1) * EMBED],
                                in_=xs_r[half * VC + c],
                            )
                        k_off = half * NB

                        def k_slice(b):
                            return ktsb[:, b * M + k_off : b * M + k_off + NB]

                    else:
                        src = (rank + j) % NCORES
                        for c in range(VC):
                            nc.gpsimd.dma_start(
                                out=vtile[:, c * EMBED : (c + 1) * EMBED],
                                in_=xv_r[
                                    ds(src * (M // P) + half * VC + c, 1)
                                ].squeeze(0),
                            )
                        ktile = kvp.tile([P, DC * NB], BF16, tag="ktile")
                        for b in range(DC):
                            nc.gpsimd.dma_start(
                                out=ktile[:, b * NB : (b + 1) * NB],
                                in_=gkt_h[half][ds(src * DC + b, 1)].squeeze(0),
                            )

                        def k_slice(b, _kt=ktile):
                            return _kt[:, b * NB : (b + 1) * NB]

                    # scores + exp: P^T[key, query] in bf16
                    pt_sb = pbp.tile([P, VC * M], BF16, tag="pt_sb")
                    for h in range(M // NB):  # query column half
                        for c in range(VC):  # key chunk within block
                            ps_s = ps_sp.tile([P, NB], FP32, tag="ps_s")
                            for b in range(DC):
                                nc.tensor.matmul(
                                    ps_s[:],
                                    lhsT=k_slice(b)[:, c * P : (c + 1) * P],
                                    rhs=qt[:, b * M + h * NB : b * M + (h + 1) * NB],
                                    start=(b == 0),
                                    stop=(b == DC - 1),
                                )
                            nc.scalar.activation(
                                out=pt_sb[:, c * M + h * NB : c * M + (h + 1) * NB],
                                in_=ps_s[:],
                                func=EXP,
                            )
                    # key-chunk sums on Vector (emitted before any PV work so
                    # the adds for both halves sit early in the DVE queue)
                    cs = []
                    for h in range(M // NB):
                        cs01 = csp.tile([P, NB], BF16, tag="cs01")
                        cs23 = csp.tile([P, NB], BF16, tag="cs23")
                        csum = csp.tile([P, NB], BF16, tag="csum")
                        nc.vector.tensor_tensor(
                            out=cs01[:],
                            in0=pt_sb[:, 0 * M + h * NB : 0 * M + (h + 1) * NB],
                            in1=pt_sb[:, 1 * M + h * NB : 1 * M + (h + 1) * NB],
                            op=ADD,
                        )
                        nc.vector.tensor_tensor(
                            out=cs23[:],
                            in0=pt_sb[:, 2 * M + h * NB : 2 * M + (h + 1) * NB],
                            in1=pt_sb[:, 3 * M + h * NB : 3 * M + (h + 1) * NB],
                            op=ADD,
                        )
                        nc.vector.tensor_tensor(
                            out=csum[:], in0=cs01[:], in1=cs23[:], op=ADD
                        )
                        cs.append(csum)

                    def sums_pass(h):
                        # replicated partition-sum via ones-vector matmul
                        ps_sum = ps_up.tile([P, NB], FP32, tag="ps_sum")
                        nc.tensor.matmul(
                            ps_sum[:], lhsT=ones_sb[:], rhs=cs[h][:],
                            start=True, stop=True,
                        )
                        dsts = sums_acc[:, h * NB : (h + 1) * NB]
                        if first:
                            nc.vector.tensor_copy(out=dsts, in_=ps_sum[:])
                        else:
                            nc.vector.tensor_tensor(
                                out=dsts, in0=dsts, in1=ps_sum[:], op=ADD
                            )

                    def pv_pass(mb):
                        for h in range(EMBED // NB):
                            ps_o = ps_op.tile([P, NB], FP32, tag="ps_o")
                            for t in range(VC):
                                nc.tensor.matmul(
                                    ps_o[:],
                                    lhsT=pt_sb[:, t * M + mb * P : t * M + (mb + 1) * P],
                                    rhs=vtile[:, t * EMBED + h * NB : t * EMBED + (h + 1) * NB],
                                    start=(t == 0),
                                    stop=(t == VC - 1),
                                )
                            dst = out_acc[:, mb * EMBED + h * NB : mb * EMBED + (h + 1) * NB]
                            if first:
                                nc.vector.tensor_copy(out=dst, in_=ps_o[:])
                            else:
                                nc.vector.tensor_tensor(
                                    out=dst, in0=dst, in1=ps_o[:], op=ADD
                                )

                    def finalize(mb):
                        # divide by softmax sum, write out (overlaps later PV);
                        # on ScalarE (idle by now) to keep the DVE tail short
                        outf = fin.tile([P, EMBED], BF16, tag="outf")
                        nc.scalar.activation(
                            out=outf[:],
                            in_=out_acc[:, mb * EMBED : (mb + 1) * EMBED],
                            func=mybir.ActivationFunctionType.Copy,
                            scale=rtot[:, mb : mb + 1],
                        )
                        nc.sync.dma_start(out=out_r[mb], in_=outf[:])

                    # PE order keeps the sums matmuls behind enough PV work
                    # that their Vector-side inputs are always ready.
                    sums_pass(0)
                    pv_pass(0)
                    pv_pass(1)
                    sums_pass(1)
                    if last:
                        # sums_acc final: per-query reciprocal while PV of the
                        # remaining query blocks still runs on PE.
                        for mb in range(MB):
                            ps_f = ps_up.tile([P, P], FP32, tag="ps_f", bufs=1)
                            nc.tensor.transpose(
                                out=ps_f[:],
                                in_=sums_acc[:, mb * P : (mb + 1) * P],
                                identity=ident[:],
                            )
                            nc.vector.tensor_copy(
                                out=scol[:, mb : mb + 1], in_=ps_f[:, 0:1]
                            )
                        nc.vector.reciprocal(out=rtot[:], in_=scol[:])
                        finalize(0)
                        finalize(1)
                    for mb in range(2, MB):
                        pv_pass(mb)
                        if last:
                            finalize(mb)

    nc.compile()
    return nc


_NC = None


def _get_nc():
    global _NC
    if _NC is None:
        _NC = _build()
    return _NC


def _run(x, rotation_params, entangle_params, **spmd_kwargs):
    import ml_dtypes

    BF = ml_dtypes.bfloat16
    x = np.ascontiguousarray(np.asarray(x, dtype=np.float32))
    wq = np.asarray(rotation_params, dtype=np.float32).reshape(EMBED, EMBED) * np.float32(
        1.0 / 32.0
    )
    wk = np.asarray(entangle_params, dtype=np.float32).reshape(EMBED, EMBED)
    xt_bf = np.ascontiguousarray(x.T).astype(BF)
    x_bf = x.astype(BF)
    wq_bf = wq.astype(BF)
    wk_bf = wk.astype(BF)
    in_maps = [
        {
            "xt_shard": np.ascontiguousarray(xt_bf[:, i * M : (i + 1) * M]),
            "x_shard": np.ascontiguousarray(x_bf[i * M : (i + 1) * M]),
            "x_full": x_bf,
            "wq": wq_bf,
            "wk": wk_bf,
        }
        for i in range(NCORES)
    ]
    res = bass_utils.run_bass_kernel_spmd(
        _get_nc(), in_maps, core_ids=list(range(NCORES)), **spmd_kwargs
    )
    out = np.concatenate(
        [res.results[i]["out"].astype(np.float32) for i in range(NCORES)], axis=0
    )
    return out, res


def kernel(x, rotation_params, entangle_params):
    out, _ = _run(x, rotation_params, entangle_params)
    return out


# revision 9
# speedup vs baseline: 1.0920x; 1.0023x over previous
"""Trainium2 Bass kernel for ClassicalSelfAttention.

  out = softmax((x @ Wq) @ (x @ Wk)^T / sqrt(D)) @ x      x: [8192, 1024] f32

Sharding (8 NeuronCores): rows of x are sharded across cores; each core
projects its own row-shard to Q^T and K^T, the K^T shards are AllGathered
across cores in two halves along the key dim (each fired as soon as its
half of the K projection finishes, so the collective overlaps the rest of
the projection and the own-block compute), and each core runs a streaming
attention loop over 16 key-blocks of 512 keys: scores matmul -> exp on
ScalarE -> PV matmul accumulated in SBUF. The softmax division is folded
into the final output scale. 1/sqrt(1024) = 2^-5 is folded into Wq on the
host (exact). All matmuls run in bf16 with fp32 PSUM accumulation
(measured end-to-end rel err ~4e-3, well within tolerance); bf16 halves
the input DMA so the PE starts sooner.

The scores matmul keeps K^T stationary and Q^T moving, so PSUM holds
scores TRANSPOSED ([key, query]); exp of that is P^T directly -- exactly
the layout the PV matmul needs as its stationary operand -- so no PE
transposes are needed. The softmax row-sums (a partition-dim reduction in
this layout) are computed by first summing the four 128-key chunks on the
Vector engine and then a single ones-vector matmul per query half (2
matmuls per block instead of 8, saving ~25us of PE time); the per-block
emission order (scores, sums_h0, PV mb0, PV mb1, sums_h1, PV mb2..7)
keeps the PE from ever waiting on the Vector-engine chunk sums.

Block processing order: each core processes its OWN two key blocks first
straight out of SBUF (plus its own V rows from a per-core x_shard input),
then the half-0 key blocks of peers (rank+1..rank+7, via dynamic DMA
offsets), then the half-1 blocks; softmax over key blocks is
order-invariant so any order works. This gives the half-0 AllGather
~150us of compute cover and the half-1 AllGather ~380us.

The final division by the softmax sums is fused into the last key block:
the sums transposes/reciprocal run on PE/Vector between that block's PV
matmuls, and each query-block's scaled output (computed on the otherwise
idle ScalarE, bf16) DMAs out while the PE is still working on later
query blocks, removing the serial tail.

A scheduler-only fence (tc.no_sync_barrier) at the top of every key
block stops the tile scheduler from hoisting gather-dependent work of a
later block ahead of gather-free work in any engine stream; without it,
large cross-core launch skew (the collective barrier rendezvous varies
~35-110us run to run) stalled even the own-key blocks on the AllGather.

Measured: ~640us (1.11x over the 711us predecessor), PE busy ~612us at
the hardware's GPIO-throttled matmul issue rate (262ns per 512-wide
matmul = 81.25% duty), i.e. the kernel sits at the throttled tensor
roofline; remaining overhead is ~12us DMA spin-up/startup and ~9us
drain tail.
"""

import sys

import numpy as np

try:
    import concourse.bass as bass  # noqa: F401
except ImportError:  # pragma: no cover
    sys.path.insert(0, "/opt/trn_rl_repo")

import concourse.bacc as bacc
import concourse.mybir as mybir
import concourse.tile as tile
from concourse.masks import make_identity
from concourse import bass_utils
from concourse.bass import ds

N_TOKENS = 8192
EMBED = 1024
NCORES = 8
M = N_TOKENS // NCORES  # rows per core (1024)
P = 128  # partitions
DC = EMBED // P  # contraction chunks (8)
NB = 512  # key-block width
NNB = N_TOKENS // NB  # key blocks (16)
MB = M // P  # query row-blocks per core (8)
VC = NB // P  # value chunks per key block (4)
HPR = M // NB  # key-block halves per rank (2)
FP32 = mybir.dt.float32
BF16 = mybir.dt.bfloat16
EXP = mybir.ActivationFunctionType.Exp
ADD = mybir.AluOpType.add
AXX = mybir.AxisListType.X


def _build():
    nc = bacc.Bacc(
        "TRN2", target_bir_lowering=False, debug=False, num_devices=NCORES
    )
    xt_shard = nc.dram_tensor("xt_shard", [EMBED, M], BF16, kind="ExternalInput").ap()
    x_shard = nc.dram_tensor("x_shard", [M, EMBED], BF16, kind="ExternalInput").ap()
    x_full = nc.dram_tensor(
        "x_full", [N_TOKENS, EMBED], BF16, kind="ExternalInput"
    ).ap()
    wq_d = nc.dram_tensor("wq", [EMBED, EMBED], BF16, kind="ExternalInput").ap()
    wk_d = nc.dram_tensor("wk", [EMBED, EMBED], BF16, kind="ExternalInput").ap()
    out_d = nc.dram_tensor("out", [M, EMBED], BF16, kind="ExternalOutput").ap()

    wq_r = wq_d.rearrange("(a p) d -> a p d", p=P)  # [DC, P, EMBED]
    wk_r = wk_d.rearrange("(a p) d -> a p d", p=P)
    xt_r = xt_shard.rearrange("(a p) m -> a p m", p=P)  # [DC, P, M]
    xs_r = x_shard.rearrange("(t p) d -> t p d", p=P)  # [M//P, P, EMBED]
    xv_r = x_full.rearrange("(t p) d -> t p d", p=P)  # [64, P, EMBED]
    out_r = out_d.rearrange("(t p) d -> t p d", p=P)  # [MB, P, EMBED]

    with tile.TileContext(nc) as tc:
        with (
            tc.tile_pool(name="persist", bufs=1) as pers,
            tc.tile_pool(name="persist_dram", bufs=1, space="DRAM") as pdram,
        ):
            ones_sb = pers.tile([P, P], BF16)
            nc.vector.memset(ones_sb[:], 1.0)
            ident = pers.tile([P, P], FP32)
            make_identity(nc, ident[:])
            # Q^T resident for the whole kernel: qt[p, b*M + m] = Qt[b*128+p, m]
            qt = pers.tile([P, DC * M], BF16)
            # own K^T shard, kept resident: ktsb[p, b*M + n] = Kt_own[b*128+p, n]
            ktsb = pers.tile([P, DC * M], BF16)
            # fp32 PV accumulator per query block: [p, mb*EMBED + dv]
            out_acc = pers.tile([P, MB * EMBED], FP32)
            # softmax denominators, replicated across partitions: [p, m]
            sums_acc = pers.tile([P, M], FP32)
            # per-query reciprocal denominators: [p, mb] for query mb*128+p
            scol = pers.tile([P, MB], FP32)
            rtot = pers.tile([P, MB], FP32)
            # K^T shard halves (AllGather inputs) and gathered halves.
            # ktd_h[j][b] = own K^T rows b*128..(b+1)*128-1, keys half j.
            # gkt_h[j][r*DC + b] = same for rank r.
            ktd_h = [
                pdram.tile([DC, P, NB], BF16, name=f"ktd{j}", tag=f"ktd{j}")
                for j in range(HPR)
            ]
            gkt_h = [
                pdram.tile(
                    [NCORES * DC, P, NB],
                    BF16,
                    addr_space="Shared",
                    name=f"gkt{j}",
                    tag=f"gkt{j}",
                )
                for j in range(HPR)
            ]

            rank = nc.gpsimd.partition_id()

            # ---- Phase A: project K^T shard (AllGather each key half as it
            # completes), then Q^T (own rows).
            with (
                tc.tile_pool(name="proj", bufs=1) as proj,
                tc.tile_pool(name="proj_ps", bufs=4, space="PSUM") as proj_ps,
            ):
                wq_sb = proj.tile([P, DC * EMBED], BF16)
                wk_sb = proj.tile([P, DC * EMBED], BF16)
                xt_sb = proj.tile([P, DC * M], BF16)
                # DMA priority: wk + the j=0 key-half of xt (what the first
                # K-proj matmuls consume), then xt's j=1 half, then wq.
                for a in range(DC):
                    nc.sync.dma_start(
                        out=wk_sb[:, a * EMBED : (a + 1) * EMBED], in_=wk_r[a]
                    )
                    nc.sync.dma_start(
                        out=xt_sb[:, a * M : a * M + NB], in_=xt_r[a][:, 0:NB]
                    )
                for a in range(DC):
                    nc.sync.dma_start(
                        out=xt_sb[:, a * M + NB : (a + 1) * M], in_=xt_r[a][:, NB:M]
                    )
                for a in range(DC):
                    nc.sync.dma_start(
                        out=wq_sb[:, a * EMBED : (a + 1) * EMBED], in_=wq_r[a]
                    )
                # K^T first so its AllGathers overlap the Q^T projection;
                # key-half j outer so each half's gather fires at the
                # halfway point of the K projection.
                for j in range(HPR):
                    for b in range(DC):  # output dim chunk
                        ps = proj_ps.tile([P, NB], FP32, tag="proj_ps")
                        for a in range(DC):  # contraction chunk
                            nc.tensor.matmul(
                                ps[:],
                                lhsT=wk_sb[:, a * EMBED + b * P : a * EMBED + (b + 1) * P],
                                rhs=xt_sb[:, a * M + j * NB : a * M + (j + 1) * NB],
                                start=(a == 0),
                                stop=(a == DC - 1),
                            )
                        nc.vector.tensor_copy(
                            out=ktsb[:, b * M + j * NB : b * M + (j + 1) * NB],
                            in_=ps[:],
                        )
                        nc.sync.dma_start(
                            out=ktd_h[j][b],
                            in_=ktsb[:, b * M + j * NB : b * M + (j + 1) * NB],
                        )
                    nc.gpsimd.collective_compute(
                        "AllGather",
                        mybir.AluOpType.bypass,
                        replica_groups=[list(range(NCORES))],
                        ins=[ktd_h[j].opt()],
                        outs=[gkt_h[j].opt()],
                    )
                for b in range(DC):
                    for j in range(HPR):
                        ps = proj_ps.tile([P, NB], FP32, tag="proj_ps")
                        for a in range(DC):
                            nc.tensor.matmul(
                                ps[:],
                                lhsT=wq_sb[:, a * EMBED + b * P : a * EMBED + (b + 1) * P],
                                rhs=xt_sb[:, a * M + j * NB : a * M + (j + 1) * NB],
                                start=(a == 0),
                                stop=(a == DC - 1),
                            )
                        nc.vector.tensor_copy(
                            out=qt[:, b * M + j * NB : b * M + (j + 1) * NB],
                            in_=ps[:],
                        )

            # ---- Phase B: streaming attention over key blocks.
            # Own two blocks first (K^T already in SBUF), then peers'
            # half-0 blocks, then peers' half-1 blocks.
            order = [(0, h) for h in range(HPR)] + [
                (j, h) for h in range(HPR) for j in range(1, NCORES)
            ]
            with (
                tc.tile_pool(name="kv", bufs=2) as kvp,
                tc.tile_pool(name="pb", bufs=3) as pbp,
                tc.tile_pool(name="cspool", bufs=2) as csp,
                tc.tile_pool(name="fin", bufs=2) as fin,
                tc.tile_pool(name="ps_s", bufs=3, space="PSUM") as ps_sp,
                tc.tile_pool(name="ps_u", bufs=2, space="PSUM") as ps_up,
                tc.tile_pool(name="ps_o", bufs=2, space="PSUM") as ps_op,
            ):
                for nb, (j, half) in enumerate(order):
                    # Scheduler-only fence (no runtime sync): keeps the tile
                    # scheduler from hoisting a later block's gather-dependent
                    # work ahead of this block in any engine stream, which
                    # would stall gather-free work on the collective when the
                    # cross-core launch skew is large.
                    tc.no_sync_barrier()
                    first, last = nb == 0, nb == NNB - 1
                    vtile = kvp.tile([P, VC * EMBED], BF16, tag="vtile")
                    if j == 0:
                        # own keys: K^T already in SBUF, V rows from x_shard
                        for c in range(VC):
                            nc.sync.dma_start(
                                out=vtile[:, c * EMBED : (c + 1) * EMBED],
                                in_=xs_r[half * VC + c],
                            )
                        k_off = half * NB

                        def k_slice(b):
                            return ktsb[:, b * M + k_off : b * M + k_off + NB]

                    else:
                        src = (rank + j) % NCORES
                        for c in range(VC):
                            nc.gpsimd.dma_start(
                                out=vtile[:, c * EMBED : (c + 1) * EMBED],
                                in_=xv_r[
                                    ds(src * (M // P) + half * VC + c, 1)
                                ].squeeze(0),
                            )
                        ktile = kvp.tile([P, DC * NB], BF16, tag="ktile")
                        for b in range(DC):
                            nc.gpsimd.dma_start(
                                out=ktile[:, b * NB : (b + 1) * NB],
                                in_=gkt_h[half][ds(src * DC + b, 1)].squeeze(0),
                            )

                        def k_slice(b, _kt=ktile):
                            return _kt[:, b * NB : (b + 1) * NB]

                    # scores + exp: P^T[key, query] in bf16
                    pt_sb = pbp.tile([P, VC * M], BF16, tag="pt_sb")
                    for h in range(M // NB):  # query column half
                        for c in range(VC):  # key chunk within block
                            ps_s = ps_sp.tile([P, NB], FP32, tag="ps_s")
                            for b in range(DC):
                                nc.tensor.matmul(
                                    ps_s[:],
                                    lhsT=k_slice(b)[:, c * P : (c + 1) * P],
                                    rhs=qt[:, b * M + h * NB : b * M + (h + 1) * NB],
                                    start=(b == 0),
                                    stop=(b == DC - 1),
                                )
                            nc.scalar.activation(
                                out=pt_sb[:, c * M + h * NB : c * M + (h + 1) * NB],
                                in_=ps_s[:],
                                func=EXP,
                            )
                    # key-chunk sums on Vector (emitted before any PV work so
                    # the adds for both halves sit early in the DVE queue)
                    cs = []
                    for h in range(M // NB):
                        cs01 = csp.tile([P, NB], BF16, tag="cs01")
                        cs23 = csp.tile([P, NB], BF16, tag="cs23")
                        csum = csp.tile([P, NB], BF16, tag="csum")
                        nc.vector.tensor_tensor(
                            out=cs01[:],
                            in0=pt_sb[:, 0 * M + h * NB : 0 * M + (h + 1) * NB],
                            in1=pt_sb[:, 1 * M + h * NB : 1 * M + (h + 1) * NB],
                            op=ADD,
                        )
                        nc.vector.tensor_tensor(
                            out=cs23[:],
                            in0=pt_sb[:, 2 * M + h * NB : 2 * M + (h + 1) * NB],
                            in1=pt_sb[:, 3 * M + h * NB : 3 * M + (h + 1) * NB],
                            op=ADD,
                        )
                        nc.vector.tensor_tensor(
                            out=csum[:], in0=cs01[:], in1=cs23[:], op=ADD
                        )
                        cs.append(csum)

                    def sums_pass(h):
                        # replicated partition-sum via ones-vector matmul
                        ps_sum = ps_up.tile([P, NB], FP32, tag="ps_sum")
                        nc.tensor.matmul(
                            ps_sum[:], lhsT=ones_sb[:], rhs=cs[h][:],
                            start=True, stop=True,
                        )
                        dsts = sums_acc[:, h * NB : (h + 1) * NB]
                        if first:
                            nc.vector.tensor_copy(out=dsts, in_=ps_sum[:])
                        else:
                            nc.vector.tensor_tensor(
                                out=dsts, in0=dsts, in1=ps_sum[:], op=ADD
                            )

                    def pv_pass(mb):
                        for h in range(EMBED // NB):
                            ps_o = ps_op.tile([P, NB], FP32, tag="ps_o")
                            for t in range(VC):
                                nc.tensor.matmul(
                                    ps_o[:],
                                    lhsT=pt_sb[:, t * M + mb * P : t * M + (mb + 1) * P],
                                    rhs=vtile[:, t * EMBED + h * NB : t * EMBED + (h + 1) * NB],
                                    start=(t == 0),
                                    stop=(t == VC - 1),
                                )
                            dst = out_acc[:, mb * EMBED + h * NB : mb * EMBED + (h + 1) * NB]
                            if first:
                                nc.vector.tensor_copy(out=dst, in_=ps_o[:])
                            else:
                                nc.vector.tensor_tensor(
                                    out=dst, in0=dst, in1=ps_o[:], op=ADD
                                )

                    def finalize(mb):
                        # divide by softmax sum, write out (overlaps later PV);
                        # on ScalarE (idle by now) to keep the DVE tail short
                        outf = fin.tile([P, EMBED], BF16, tag="outf")
                        nc.scalar.activation(
                            out=outf[:],
                            in_=out_acc[:, mb * EMBED : (mb + 1) * EMBED],
                            func=mybir.ActivationFunctionType.Copy,
                            scale=rtot[:, mb : mb + 1],
                        )
                        nc.sync.dma_start(out=out_r[mb], in_=outf[:])

                    # PE order keeps the sums matmuls behind enough PV work
                    # that their Vector-side inputs are always ready.
                    sums_pass(0)
                    pv_pass(0)
                    pv_pass(1)
                    sums_pass(1)
                    if last:
                        # sums_acc final: per-query reciprocal while PV of the
                        # remaining query blocks still runs on PE.
                        for mb in range(MB):
                            ps_f = ps_up.tile([P, P], FP32, tag="ps_f", bufs=1)
                            nc.tensor.transpose(
                                out=ps_f[:],
                                in_=sums_acc[:, mb * P : (mb + 1) * P],
                                identity=ident[:],
                            )
                            nc.vector.tensor_copy(
                                out=scol[:, mb : mb + 1], in_=ps_f[:, 0:1]
                            )
                        nc.vector.reciprocal(out=rtot[:], in_=scol[:])
                        finalize(0)
                        finalize(1)
                    for mb in range(2, MB):
                        pv_pass(mb)
                        if last:
                            finalize(mb)

    nc.compile()
    return nc


_NC = None


def _get_nc():
    global _NC
    if _NC is None:
        _NC = _build()
    return _NC


def _run(x, rotation_params, entangle_params, **spmd_kwargs):
    import ml_dtypes

    BF = ml_dtypes.bfloat16
    x = np.ascontiguousarray(np.asarray(x, dtype=np.float32))
    wq = np.asarray(rotation_params, dtype=np.float32).reshape(EMBED, EMBED) * np.float32(
        1.0 / 32.0
    )
    wk = np.asarray(entangle_params, dtype=np.float32).reshape(EMBED, EMBED)
    xt_bf = np.ascontiguousarray(x.T).astype(BF)
    x_bf = x.astype(BF)
    wq_bf = wq.astype(BF)
    wk_bf = wk.astype(BF)
    in_maps = [
        {
            "xt_shard": np.ascontiguousarray(xt_bf[:, i * M : (i + 1) * M]),
            "x_shard": np.ascontiguousarray(x_bf[i * M : (i + 1) * M]),
            "x_full": x_bf,
            "wq": wq_bf,
            "wk": wk_bf,
        }
        for i in range(NCORES)
    ]
    res = bass_utils.run_bass_kernel_spmd(
        _get_nc(), in_maps, core_ids=list(range(NCORES)), **spmd_kwargs
    )
    out = np.concatenate(
        [res.results[i]["out"].astype(np.float32) for i in range(NCORES)], axis=0
    )
    return out, res


def kernel(x, rotation_params, entangle_params):
    out, _ = _run(x, rotation_params, entangle_params)
    return out
